# revision 9
# baseline (speedup 1.0000x reference)
"""PrefSimMat (EucDis mode) Trainium2 kernel.

sim[i,j] = 1 - dist[i,j] / ||dist[i,:]||_2,  dist = pairwise Euclidean
distance of the rows of p_u [8192, 256] fp32.

Strategy (8 NeuronCores, data-parallel over query rows):
  - Each core computes a [1024, 8192] tile of the output.
  - Gram-matrix identity: sq[i,j] = ni + nj - 2*g[i,j], contracted on
    TensorE in bf16.  (fp8 was measured 2.6x SLOWER per stream on this
    part in both normal and DoubleRow modes -- the PE clock never leaves
    its cold 1.2 GHz state under fp8 -- so bf16 with Fast Weight Load is
    the fastest contraction here.)
  - PE floor is 3 streams per output column: 2 feature chunks (K=128)
    plus one K=2 extension chunk carrying the per-column nj term as
    bf16 hi/lo splits (nj - 256 = hi + lo, abs err ~4e-4).  The per-row
    terms ni + 256 + eps ride in the ScalarE activation bias, so no
    stream is spent on them (the old kernel burned a third full-rate
    128-row stream on a padded ni/nj/eps chunk).
  - Column-major loop order (column group outer, m-chunk inner): the
    first 2048-column rhs slice (~1.5 MiB with weights) is enough to
    start 20us of PE work, so the PE no longer stalls ~13us waiting for
    the full 6 MiB rhs load, and the HAM clock warms up early.
  - Row norms are computed analytically on the host (O(N*D)) from the
    quantized features, so device and host are numerically consistent:
    rowsum_i = N*(ni+eps) + sum_j nj_eff - 2 * a_i . (sum_j a_j).
  - ScalarE: t = Sqrt(psum * r2_i + r2_i*(ni+256+eps)) (per-partition
    scale/bias APs) = dist_ij/rownorm_i, written as fp16.
  - VectorE: out = t * (-1) + 1 (fp16 -> fp16, packed-2-byte fast mode).
  - Output DMA'd per (m-chunk, group) [128, 2048] fp16 slice (512 KiB)
    from a 4-deep staging ring, so the final drain is ~2us instead of
    the 6.3us a whole-chunk transfer costs.

Raw Bass (no TileContext): the walrus build in this container allows at most
one semaphore wait attached per compute instruction, so all cross-engine
dependencies are standalone wait_ge instructions with hand-rolled semaphores.
CoreSim race rule: every semaphore update crossing a waited threshold must be
ordered by its own issuing engine -> one semaphore per input DMA, and each
output staging slot gets its own semaphore with issuing-engine self-waits.
"""

import numpy as np
import ml_dtypes

BF16 = ml_dtypes.bfloat16

N = 8192        # rows of p_u == output dim
D = 256         # feature dim
P = 128         # partitions
NCORES = 8
M_PER_CORE = N // NCORES       # 1024 output rows per core
MC = M_PER_CORE // P           # 8 m-chunks of 128 rows
KE = 2          # live extension rows (nj hi/lo)
KEP = 128       # ext chunk zero-padded to a full K=128 partition chunk
                # (non-uniform K reconfigures the PE row-group mode every
                # tile -- measured: it pins the clock cold and costs ~2.6x)
NT = 512        # matmul free-dim tile (one PSUM bank fp32)
GW = 2048       # ACT/DVE group width = 4 PSUM banks
NG = N // GW    # 4 column groups
EPS = 2.0 ** -3 # keeps sqrt argument positive on the diagonal under
                # PSUM rounding (device excursions ~0.01 observed)
CNJ = 256.0     # nj centering constant (absorbed into the ACT bias)

OUT_DT = np.float16

_CACHE = {}


def _build_nc():
    import concourse.bass as bass
    import concourse.mybir as mybir

    f32 = mybir.dt.float32
    f16 = mybir.dt.float16
    bf16 = mybir.dt.bfloat16
    AF = mybir.ActivationFunctionType
    ALU = mybir.AluOpType

    nc = bass.Bass()
    l0_d = nc.dram_tensor("l0", [P, M_PER_CORE], bf16, kind="ExternalInput")
    l1_d = nc.dram_tensor("l1", [P, M_PER_CORE], bf16, kind="ExternalInput")
    r0_d = nc.dram_tensor("r0", [P, N], bf16, kind="ExternalInput")
    r1_d = nc.dram_tensor("r1", [P, N], bf16, kind="ExternalInput")
    extw_d = nc.dram_tensor("extw", [KEP, P], bf16, kind="ExternalInput")
    extr_d = nc.dram_tensor("extr", [KEP, N], bf16, kind="ExternalInput")
    sc_d = nc.dram_tensor("sc", [P, 2 * MC], f32, kind="ExternalInput")
    out_d = nc.dram_tensor("out", [M_PER_CORE, N], f16, kind="ExternalOutput")

    NGI = MC * NG  # 32 pipeline groups, order: gi = g * MC + m

    from contextlib import ExitStack

    with ExitStack() as ctx:
        r0_s = ctx.enter_context(nc.sbuf_tensor("r0_s", [P, N], bf16))
        r1_s = ctx.enter_context(nc.sbuf_tensor("r1_s", [P, N], bf16))
        l0_s = ctx.enter_context(nc.sbuf_tensor("l0_s", [P, M_PER_CORE], bf16))
        l1_s = ctx.enter_context(nc.sbuf_tensor("l1_s", [P, M_PER_CORE], bf16))
        extw_s = ctx.enter_context(nc.sbuf_tensor("extw_s", [KEP, P], bf16))
        extr_s = ctx.enter_context(nc.sbuf_tensor("extr_s", [KEP, N], bf16))
        sc_s = ctx.enter_context(nc.sbuf_tensor("sc_s", [P, 2 * MC], f32))
        tbuf = ctx.enter_context(nc.sbuf_tensor("tbuf", [P, 4 * GW], f16))
        stage = ctx.enter_context(nc.sbuf_tensor("stage", [P, 4 * GW], f16))
        ps = ctx.enter_context(nc.psum_tensor("ps", [P, 2 * GW], f32))
        rhs_g_sems = [
            [ctx.enter_context(nc.semaphore(f"in_r{c}_{g}")) for c in range(3)]
            for g in range(NG)
        ]
        in_l = ctx.enter_context(nc.semaphore("in_l"))
        in_ext = ctx.enter_context(nc.semaphore("in_ext"))
        in_sc = ctx.enter_context(nc.semaphore("in_sc"))
        sem_mm = ctx.enter_context(nc.semaphore("sem_mm"))
        sem_act = ctx.enter_context(nc.semaphore("sem_act"))
        sem_ts = ctx.enter_context(nc.semaphore("sem_ts"))
        out_sems = [ctx.enter_context(nc.semaphore(f"dma_o{s}")) for s in range(4)]
        block = ctx.enter_context(nc.Block())

        @block.gpsimd
        def _(gp):
            # input stream on the Pool engine's DMA queue, so it does not
            # serialize behind the SP output stream
            for g in range(NG):
                c0, c1 = g * GW, (g + 1) * GW
                gp.dma_start(r0_s[:, c0:c1], r0_d[:, c0:c1]).then_inc(
                    rhs_g_sems[g][0], 16
                )
                gp.dma_start(extr_s[:, c0:c1], extr_d[:, c0:c1]).then_inc(
                    rhs_g_sems[g][2], 16
                )

        @block.sync
        def _(sync):
            for g in range(NG):
                c0, c1 = g * GW, (g + 1) * GW
                sync.dma_start(r1_s[:, c0:c1], r1_d[:, c0:c1]).then_inc(
                    rhs_g_sems[g][1], 16
                )
            for gi in range(NGI):
                g, m = divmod(gi, MC)
                sync.wait_ge(sem_ts, gi + 1)
                if gi >= 4:
                    # self-serialize this staging slot's DMA stream
                    sync.wait_ge(out_sems[gi % 4], 16 * (gi // 4))
                sync.dma_start(
                    out_d[m * P : (m + 1) * P, g * GW : (g + 1) * GW],
                    stage[:, (gi % 4) * GW : (gi % 4 + 1) * GW],
                ).then_inc(out_sems[gi % 4], 16)

        @block.tensor
        def _(tensor):
            tensor.wait_ge(in_l, 32)
            tensor.wait_ge(in_ext, 16)
            for g in range(NG):
                for s in rhs_g_sems[g]:
                    tensor.wait_ge(s, 16)
                for m in range(MC):
                    gi = g * MC + m
                    lsl0 = l0_s[:, m * P : (m + 1) * P]
                    lsl1 = l1_s[:, m * P : (m + 1) * P]
                    if gi >= 2:
                        tensor.wait_ge(sem_act, gi - 1)
                    inst = None
                    for j in range(GW // NT):
                        n0 = g * GW + j * NT
                        p0 = (gi % 2) * GW + j * NT
                        tensor.matmul(
                            ps[:, p0 : p0 + NT],
                            lsl0,
                            r0_s[:, n0 : n0 + NT],
                            start=True,
                            stop=False,
                        )
                        tensor.matmul(
                            ps[:, p0 : p0 + NT],
                            lsl1,
                            r1_s[:, n0 : n0 + NT],
                            start=False,
                            stop=False,
                        )
                        inst = tensor.matmul(
                            ps[:, p0 : p0 + NT],
                            extw_s[:, :],
                            extr_s[:, n0 : n0 + NT],
                            start=False,
                            stop=True,
                        )
                    inst.then_inc(sem_mm, 1)

        @block.scalar
        def _(scalar):
            # small input tensors on the ACT queue (idle until the first
            # matmul group completes)
            scalar.dma_start(l0_s[:, :], l0_d[:, :]).then_inc(in_l, 16)
            scalar.dma_start(l1_s[:, :], l1_d[:, :]).then_inc(in_l, 16)
            scalar.dma_start(extw_s[:, :], extw_d[:, :]).then_inc(in_ext, 16)
            scalar.dma_start(sc_s[:, :], sc_d[:, :]).then_inc(in_sc, 16)
            scalar.wait_ge(in_sc, 16)
            for gi in range(NGI):
                m = gi % MC
                scalar.wait_ge(sem_mm, gi + 1)
                if gi >= 4:
                    scalar.wait_ge(sem_ts, gi - 3)
                scalar.activation(
                    tbuf[:, (gi % 4) * GW : (gi % 4 + 1) * GW],
                    ps[:, (gi % 2) * GW : (gi % 2 + 1) * GW],
                    AF.Sqrt,
                    scale=sc_s[:, m : m + 1],
                    bias=sc_s[:, MC + m : MC + m + 1],
                ).then_inc(sem_act, 1)

        @block.vector
        def _(vector):
            for gi in range(NGI):
                vector.wait_ge(sem_act, gi + 1)
                if gi >= 4:
                    vector.wait_ge(out_sems[gi % 4], 16 * (gi // 4))
                vector.tensor_scalar(
                    stage[:, (gi % 4) * GW : (gi % 4 + 1) * GW],
                    tbuf[:, (gi % 4) * GW : (gi % 4 + 1) * GW],
                    -1.0,
                    1.0,
                    op0=ALU.mult,
                    op1=ALU.add,
                ).then_inc(sem_ts, 1)

    return nc


def _prep_inputs(p_u):
    """Host-side O(N*D) prep: bf16 cast/transpose, norms, row sums."""
    a16 = p_u.astype(BF16)                    # quantize features once
    af = a16.astype(np.float32)
    a64 = af.astype(np.float64)
    ni64 = np.einsum("ij,ij->i", a64, a64)    # [N] norms of quantized rows

    # nj extension rows: nj - CNJ = hi + lo (bf16 splits)
    njp = ni64 - CNJ
    hi16 = njp.astype(np.float32).astype(BF16)
    hi = hi16.astype(np.float64)
    lo16 = (njp - hi).astype(np.float32).astype(BF16)
    lo = lo16.astype(np.float64)
    nj_eff = CNJ + hi + lo

    t64 = a64.sum(axis=0)                     # [D]
    rowsum = N * ni64 + nj_eff.sum() - 2.0 * (a64 @ t64) + N * EPS
    r2 = 1.0 / rowsum                         # [N] f64
    bias64 = r2 * (ni64 + CNJ + EPS)

    aT16 = np.ascontiguousarray(a16.T)        # [256, 8192] bf16
    r0 = aT16[0:P]
    r1 = aT16[P : 2 * P]
    extr = np.zeros((KEP, N), dtype=BF16)
    extr[0] = hi16
    extr[1] = lo16
    extw = np.zeros((KEP, P), dtype=BF16)
    extw[0:KE, :] = BF16(1.0)

    m2 = (-2.0 * af).astype(BF16)             # exact bf16 doubling
    m2T = np.ascontiguousarray(m2.T)          # [256, 8192]
    r2f = r2.astype(np.float32)
    biasf = bias64.astype(np.float32)

    in_maps = []
    for c in range(NCORES):
        sl = slice(c * M_PER_CORE, (c + 1) * M_PER_CORE)
        l0 = np.ascontiguousarray(m2T[0:P, sl])
        l1 = np.ascontiguousarray(m2T[P : 2 * P, sl])
        sc = np.concatenate(
            [
                np.ascontiguousarray(r2f[sl].reshape(MC, P).T),
                np.ascontiguousarray(biasf[sl].reshape(MC, P).T),
            ],
            axis=1,
        ).astype(np.float32)                  # [128, 16]
        in_maps.append(
            {"l0": l0, "l1": l1, "r0": r0, "r1": r1,
             "extw": extw, "extr": extr, "sc": sc}
        )
    return in_maps


def kernel(p_u):
    from concourse.bass_utils import run_bass_kernel_spmd

    p_u = np.asarray(p_u, dtype=np.float32)
    assert p_u.shape == (N, D)

    if "nc" not in _CACHE:
        _CACHE["nc"] = _build_nc()
    nc = _CACHE["nc"]

    in_maps = _prep_inputs(p_u)
    trace = bool(_CACHE.get("trace"))
    res = run_bass_kernel_spmd(nc, in_maps, core_ids=list(range(NCORES)), trace=trace)
    _CACHE["last_result"] = res
    out = np.concatenate(
        [res.results[c]["out"].astype(np.float32) for c in range(NCORES)], axis=0
    )
    return out
